# revision 19
# baseline (speedup 1.0000x reference)
"""CTRNN policy kernel for Trainium2 (8 NeuronCores, batch-parallel).

Reference computation (per batch element b, B=64, N=1024, OBS=64, A=16):
    I = E[b] @ obs[b]
    repeat int(1.0//0.1)=9 times:
        y = tanh(gain*(v+bias))*mask
        v = (v + DT/tau * (-v + W[b]@y + I)) * mask
    action[b] = D[b] @ v

Sharding: batch 64 -> 8 cores x 8 individuals, fully data parallel.

Per-core algebra (host-folded, mask/tau folded into the coefficients):
    am = DT/tau*mask, cm = (1-DT/tau)*mask
    Wf = diag(am) @ W @ diag(mask);  Ef = diag(am) @ E;  bc = bias*(1-cm)
    state vs = v + bias:
        y   = tanh(g * vs)
        vs' = cm*vs + Wf@y + (Ef@obs + bc)
    action = D @ (vs - bias)

Device mapping per individual (N=1024 split as n = p*8 + c, p=partition):
  - W^T slabs [128, 8192] bf16 resident in SBUF for all 8 individuals.
  - matvec on TensorE: stationary = y column chunk [128,1] (bf16), moving =
    W^T slab. COLGROUPS=1: 16 x N=512 matmuls into a PSUM row [1,1024].
    COLGROUPS=2/4: column-group tiling (tile_position=(0,32j)) runs 2/4
    concurrent streams on separate PSUM banks/partitions.
  - evacuate PSUM row(s) to SBUF (scalar/vector copies), DMA-scatter
    [rows] -> [128,8] column layout (ACT-ring HWDGE, separate FIFO from the
    bulk W loads on the SP ring), then cheap [128,8] vector ops + tanh.
"""

import os
import sys
from contextlib import ExitStack

import numpy as np

for _p in ("/opt/trn_rl_repo", "/root/.axon_site/_ro/trn_rl_repo"):
    if os.path.isdir(_p) and _p not in sys.path:
        sys.path.append(_p)

import ml_dtypes  # noqa: E402

import concourse.bass as bass  # noqa: E402
import concourse.tile as tile  # noqa: E402
from concourse import bacc, mybir  # noqa: E402
from concourse.bass_utils import run_bass_kernel_spmd  # noqa: E402

DT = 0.1
ITERS = int(1.0 // DT)  # == 9: 1.0//0.1 == 9.0 in fp
B_FULL, N, OBS, ADIM = 64, 1024, 64, 16
NCORES = 8
BPC = B_FULL // NCORES
P, CN = 128, 8          # n = p*8 + c
F32 = mybir.dt.float32
BF16 = mybir.dt.bfloat16
BF16_NP = ml_dtypes.bfloat16

COLGROUPS = int(os.environ.get("CTRNN_COLGROUPS", "1"))
assert COLGROUPS in (1, 2, 4)
NSLAB = N // COLGROUPS
# matmul sub-slabs: (tile_col, psum_off, w_off, width) per column group
if COLGROUPS == 1:
    GSLABS = [(0, 0, 0, 512), (0, 512, 512, 512)]
    PSW = 1024
    PS_BUFS = 3
elif COLGROUPS == 2:
    GSLABS = [(0, 0, 0, 512), (32, 512, 512, 512)]
    PSW = 1024
    PS_BUFS = 3
else:
    # all 4 column groups share one PSUM bank: disjoint partitions {0,32,64,96},
    # same free range [0:256) -> single-bank tiles, deep pipeline, 1-copy evac
    GSLABS = [(32 * j, 0, 256 * j, 256) for j in range(4)]
    PSW = 512
    PS_BUFS = 8
# consts packing: per individual 5 blocks of 8 cols: cm, g, bias, vs0, bc
NCONST = 5


def wave_schedule(iters=ITERS, mv_us=None):
    """Greedy longest-queue-first rounds honoring estimated W arrival.

    Returns a flat list of (b, t) matvec work items. Width grows as W tiles
    land (~4.8us each after W0+bulk), then stays 8-wide so the per-individual
    update chain hides behind the other individuals' PE work.
    """
    if mv_us is None:
        mv_us = 2.0 if COLGROUPS == 4 else 3.6
    w_avail = [6.0] + [12.0 + 4.8 * b for b in range(1, BPC)]
    t = 8.0
    remaining = [iters] * BPC
    order = []
    while any(remaining):
        active = [b for b in range(BPC) if remaining[b] and w_avail[b] <= t]
        if not active:
            t = min(w_avail[b] for b in range(BPC) if remaining[b])
            continue
        # longest-queue-first with a width cap: late individuals catch up so
        # all 8 finish together (no narrow chain-bound tail)
        active.sort(key=lambda b: (-remaining[b], b))
        active = active[:6]
        order.append([(b, iters - remaining[b]) for b in active])
        for b in active:
            remaining[b] -= 1
        # narrow rounds are chain-latency-bound, not PE-bound
        t += max(mv_us * len(active), 8.0 if len(active) < 4 else 0.0)
    return order


def cidx(b, k):
    return (b * NCONST + k) * CN


def make_pools(ctx, tc):
    return dict(
        const=ctx.enter_context(tc.tile_pool(name="const", bufs=1)),
        wpool=ctx.enter_context(tc.tile_pool(name="w", bufs=BPC)),
        row=ctx.enter_context(tc.tile_pool(name="row", bufs=3)),
        ucol=ctx.enter_context(tc.tile_pool(name="ucol", bufs=8)),
        tmp=ctx.enter_context(tc.tile_pool(name="tmp", bufs=8)),
        ps=ctx.enter_context(tc.tile_pool(name="ps", bufs=PS_BUFS, space="PSUM")),
    )


def kernel_body(ctx, tc, ins, out_ap, iters=ITERS, pools=None):
    nc = tc.nc
    Tanh = mybir.ActivationFunctionType.Tanh
    add = mybir.AluOpType.add
    mult = mybir.AluOpType.mult
    sub = mybir.AluOpType.subtract

    p = pools if pools is not None else make_pools(ctx, tc)
    const, wpool, row, ucol, tmp, ps = (
        p["const"], p["wpool"], p["row"], p["ucol"], p["tmp"], p["ps"])

    # ---- bulk loads on the SP (sync) HWDGE ring: W first ----
    w_sb = []
    for b in range(BPC):
        w_sb.append(wpool.tile([P, CN * N], BF16, tag="w", name=f"w{b}"))
    consts = const.tile([P, BPC * NCONST * CN], F32, tag="consts", name="consts")
    nc.sync.dma_start(consts[:], ins["consts"][:])
    nc.sync.dma_start(w_sb[0][:], ins["WT"][0])
    et_sb = const.tile([OBS, BPC * N], BF16, tag="et", name="et")
    nc.sync.dma_start(et_sb[:], ins["ETall"][:])
    obs_sb = const.tile([OBS, BPC], BF16, tag="obs", name="obs")
    nc.sync.dma_start(obs_sb[:], ins["obsT"][:])
    nc.sync.dma_start(w_sb[1][:], ins["WT"][1])
    nc.sync.dma_start(w_sb[2][:], ins["WT"][2])
    dt_sb = const.tile([P, BPC * CN * ADIM], F32, tag="dt", name="dt")
    nc.sync.dma_start(dt_sb[:], ins["DTall"][:])
    for b in range(3, BPC):
        nc.sync.dma_start(w_sb[b][:], ins["WT"][b])

    # ---- per-individual state ----
    vs_sb, y_sb, icol_sb = [], [], []
    for b in range(BPC):
        vs_sb.append(const.tile([P, CN], F32, tag=f"vs{b}", name=f"vs{b}"))
        y_sb.append(const.tile([P, CN], BF16, tag=f"y{b}", name=f"y{b}"))
        icol_sb.append(const.tile([P, CN], F32, tag=f"ic{b}", name=f"ic{b}"))
    act_sb = const.tile([1, BPC * ADIM], F32, tag="act", name="act")

    def cm_ap(b):
        return consts[:, cidx(b, 0):cidx(b, 0) + CN]

    def g_ap(b):
        return consts[:, cidx(b, 1):cidx(b, 1) + CN]

    def bias_ap(b):
        return consts[:, cidx(b, 2):cidx(b, 2) + CN]

    def vs0_ap(b):
        return consts[:, cidx(b, 3):cidx(b, 3) + CN]

    def bc_ap(b):
        return consts[:, cidx(b, 4):cidx(b, 4) + CN]

    # ---- setup: I = Ef@obs (+bc) into column layout; y0 = tanh(g*vs0) ----
    for b in range(BPC):
        ir = row.tile([1, N], F32, tag="irow", name=f"ir{b}")
        for h in range(2):
            ip = ps.tile([P, PSW], F32, tag="ps", name=f"ip{b}_{h}")
            nc.tensor.matmul(
                ip[0:1, 0:512],
                obs_sb[:, b:b + 1],
                et_sb[:, b * N + h * 512: b * N + h * 512 + 512],
                start=True, stop=True,
            )
            nc.scalar.copy(ir[0:1, h * 512:(h + 1) * 512], ip[0:1, 0:512])
        itmp = ucol.tile([P, CN], F32, tag="ucol", name=f"it{b}")
        nc.scalar.dma_start(itmp[:], ir[:])  # [1,1024] -> [128,8]
        nc.vector.tensor_tensor(icol_sb[b][:], itmp[:], bc_ap(b), op=add)
        # vs = vs0; y0 = tanh(g*vs0)
        nc.vector.tensor_copy(vs_sb[b][:], vs0_ap(b))
        s0 = tmp.tile([P, CN], F32, tag="s", name=f"s0{b}")
        nc.vector.tensor_tensor(s0[:], g_ap(b), vs0_ap(b), op=mult)
        nc.scalar.activation(y_sb[b][:], s0[:], Tanh)

    # ---- recurrent loop ----
    # matvec is split: the MM/evac/scatter half is issued for a whole round
    # of individuals before any update chains, so the in-order Vector/Scalar
    # FIFOs never park an evacuation copy behind a scatter-completion wait
    # (that coupling serialized matvecs at ~2.9us regardless of PE speed)
    t1_sb, ucl_sb = {}, {}

    def matvec_mm(b, t):
        # cm*vs only needs last iteration's vs -- issue before the matmuls so
        # the post-scatter dependency chain is as short as possible
        t1 = tmp.tile([P, CN], F32, tag="t1", name="t1")
        nc.vector.tensor_tensor(t1[:], cm_ap(b), vs_sb[b][:], op=mult)
        t1_sb[b] = t1
        wy = ps.tile([P, PSW], F32, tag="ps", name="wy")
        for c in range(CN):
            yc = y_sb[b][:, c:c + 1]
            for (tcol, poff, woff, width) in GSLABS:
                nc.tensor.matmul(
                    wy[tcol:tcol + 1, poff:poff + width],
                    yc,
                    w_sb[b][:, c * N + woff: c * N + woff + width],
                    start=(c == 0), stop=(c == CN - 1),
                    tile_position=(0, tcol) if COLGROUPS > 1 else None,
                )
        u4 = row.tile([P, NSLAB] if COLGROUPS > 1 else [1, N], F32,
                      tag="u4", name="u4")
        if COLGROUPS == 4:
            # all groups live in one bank at partitions {0,32,64,96}; one
            # 128-lane copy evacuates them all (garbage lanes are unused)
            nc.vector.tensor_copy(u4[:, :], wy[:, 0:NSLAB])
        else:
            for i, (tcol, poff, woff, width) in enumerate(GSLABS):
                src = wy[tcol:tcol + 1, poff:poff + width]
                if COLGROUPS == 1:
                    dst = u4[0:1, woff:woff + width]
                else:
                    dst = u4[tcol:tcol + 1, 0:width]
                if i % 2 == 0:
                    nc.scalar.copy(dst, src)
                else:
                    nc.vector.tensor_copy(dst, src)
        ucl = ucol.tile([P, CN], F32, tag="ucol", name="u")
        if COLGROUPS == 1:
            scat_src = u4[0:1, :]
        else:
            scat_src = u4[0:32 * COLGROUPS:32, :]
        nc.scalar.dma_start(ucl[:], scat_src)
        ucl_sb[b] = ucl

    def matvec_upd(b, t):
        t0 = tmp.tile([P, CN], F32, tag="t0", name="t0")
        nc.vector.tensor_tensor(t0[:], ucl_sb[b][:], icol_sb[b][:], op=add)
        nc.vector.tensor_tensor(vs_sb[b][:], t0[:], t1_sb[b][:], op=add)
        if t < iters - 1:
            s = tmp.tile([P, CN], F32, tag="s", name="s")
            nc.vector.tensor_tensor(s[:], g_ap(b), vs_sb[b][:], op=mult)
            nc.scalar.activation(y_sb[b][:], s[:], Tanh)

    vf_sb = {}

    def decode_pre(b):
        # issued right after b's last matvec: the subtract completes long
        # before the end-of-kernel decode matmuls, so they never stall PE
        vf_sb[b] = const.tile([P, CN], F32, tag=f"vf{b}", name=f"vf{b}")
        nc.vector.tensor_tensor(vf_sb[b][:], vs_sb[b][:], bias_ap(b), op=sub)

    def decode(b):
        vf = vf_sb[b]
        ap = ps.tile([P, PSW], F32, tag="ps", name="dec")
        for c in range(CN):
            nc.tensor.matmul(
                ap[0:1, 0:ADIM],
                vf[:, c:c + 1],
                dt_sb[:, b * CN * ADIM + c * ADIM: b * CN * ADIM + (c + 1) * ADIM],
                start=(c == 0), stop=(c == CN - 1),
            )
        nc.vector.tensor_copy(act_sb[0:1, b * ADIM:(b + 1) * ADIM], ap[0:1, 0:ADIM])

    for round_items in wave_schedule(iters):
        for b, t in round_items:
            matvec_mm(b, t)
        for b, t in round_items:
            matvec_upd(b, t)
            if t == iters - 1:
                decode_pre(b)
    for b in range(BPC):
        decode(b)
    nc.sync.dma_start(out_ap[:], act_sb[0:1, :])


def build_nc(iters=ITERS):
    nc = bacc.Bacc(
        "TRN2", target_bir_lowering=False, debug=False, enable_asserts=False,
    )
    ins = {}
    ins["WT"] = nc.dram_tensor("WT", [BPC, P, CN * N], BF16, kind="ExternalInput").ap()
    ins["ETall"] = nc.dram_tensor("ETall", [OBS, BPC * N], BF16, kind="ExternalInput").ap()
    ins["obsT"] = nc.dram_tensor("obsT", [OBS, BPC], BF16, kind="ExternalInput").ap()
    ins["consts"] = nc.dram_tensor(
        "consts", [P, BPC * NCONST * CN], F32, kind="ExternalInput").ap()
    ins["DTall"] = nc.dram_tensor(
        "DTall", [P, BPC * CN * ADIM], F32, kind="ExternalInput").ap()
    out_ap = nc.dram_tensor("act", [BPC, ADIM], F32, kind="ExternalOutput").ap()

    with tile.TileContext(nc) as tc:
        with ExitStack() as ctx:
            pools = make_pools(ctx, tc)
            kernel_body(ctx, tc, ins, out_ap, iters, pools)
    nc.compile()
    return nc


def prep_in_maps(obs, v0, tau, gain, bias, W, mask, E, D):
    f = np.float32
    obs, v0, tau, gain, bias, W, mask, E, D = [
        np.asarray(x, dtype=f) for x in (obs, v0, tau, gain, bias, W, mask, E, D)
    ]
    am = (DT / tau) * mask                    # [64, N]
    cm = (1.0 - DT / tau) * mask
    Wf = W * am[:, :, None] * mask[:, None, :]
    WT = np.ascontiguousarray(Wf.transpose(0, 2, 1)).reshape(
        B_FULL, P, CN * N).astype(BF16_NP)
    ETp = np.ascontiguousarray(
        (E * am[:, :, None]).transpose(0, 2, 1)).astype(BF16_NP)  # [64, OBS, N]
    DTp = np.ascontiguousarray(D.transpose(0, 2, 1)).reshape(B_FULL, P, CN * ADIM)
    obsT = np.ascontiguousarray(obs.T).astype(BF16_NP)  # [OBS, 64]
    vs0 = (v0 + bias).reshape(B_FULL, P, CN)
    cmS = cm.reshape(B_FULL, P, CN)
    gS = gain.reshape(B_FULL, P, CN)
    bS = bias.reshape(B_FULL, P, CN)
    bcS = (bias * (1.0 - cm)).reshape(B_FULL, P, CN)

    in_maps = []
    for core in range(NCORES):
        s = slice(core * BPC, (core + 1) * BPC)
        # consts [128, BPC*5*8]: per b: cm, g, bias, vs0, bc
        cst = np.empty((P, BPC * NCONST * CN), f)
        for i, b in enumerate(range(core * BPC, (core + 1) * BPC)):
            for k, arr in enumerate((cmS, gS, bS, vs0, bcS)):
                cst[:, (i * NCONST + k) * CN:(i * NCONST + k + 1) * CN] = arr[b]
        et = np.ascontiguousarray(
            ETp[s].transpose(1, 0, 2).reshape(OBS, BPC * N))
        dtall = np.ascontiguousarray(
            DTp[s].transpose(1, 0, 2).reshape(P, BPC * CN * ADIM))
        in_maps.append({
            "WT": np.ascontiguousarray(WT[s]),
            "ETall": et,
            "obsT": np.ascontiguousarray(obsT[:, s]),
            "consts": cst,
            "DTall": dtall,
        })
    return in_maps


_NC_CACHE = None


def _get_nc():
    global _NC_CACHE
    if _NC_CACHE is None:
        _NC_CACHE = build_nc()
    return _NC_CACHE


def kernel(obs, v0, tau, gain, bias, W, mask, E, D):
    nc = _get_nc()
    in_maps = prep_in_maps(obs, v0, tau, gain, bias, W, mask, E, D)
    res = run_bass_kernel_spmd(nc, in_maps, core_ids=list(range(NCORES)))
    return np.concatenate([res.results[c]["act"] for c in range(NCORES)], axis=0)


# revision 23
# speedup vs baseline: 1.2388x; 1.2388x over previous
"""CTRNN policy kernel for Trainium2 (8 NeuronCores, batch-parallel).

Reference computation (per batch element b, B=64, N=1024, OBS=64, A=16):
    I = E[b] @ obs[b]
    repeat int(1.0//0.1)=9 times:
        y = tanh(gain*(v+bias))*mask
        v = (v + DT/tau * (-v + W[b]@y + I)) * mask
    action[b] = D[b] @ v

Sharding: batch 64 -> 8 cores x 8 individuals, fully data parallel.

Per-core algebra (host-folded, mask/tau folded into the coefficients):
    am = DT/tau*mask, cm = (1-DT/tau)*mask
    Wf = diag(am) @ W @ diag(mask);  Ef = diag(am) @ E;  bc = bias*(1-cm)
    state vs = v + bias:
        y   = tanh(g * vs)
        vs' = cm*vs + Wf@y + (Ef@obs + bc)
    action = D @ (vs - bias)

Device mapping per individual (N=1024 as n = p*8 + c for the matmul
contraction; W^T slabs [128, 8192] bf16 all resident in SBUF):

  - matvec on TensorE with 4-way column-group tiling: stationary = y column
    chunk [128,1] bf16 at array column 32j, moving = W^T n-slab [128,256].
    The 4 groups stream concurrently (separate XBUSes) and land in ONE
    shared PSUM bank at partitions {0,32,64,96} (disjoint per-partition
    accumulators), so a matvec costs ~1.9us of PE instead of ~3.5us.
  - the leak/gate update runs in "row space" [128,256] right out of PSUM
    (rows 32j hold dv n-slab j; other lanes carry zeros): tensor_tensor ops
    are lane-parallel so the garbage lanes are free. The only partition
    redistribution is the y scatter [4x256 rows] -> [128,8] bf16 column
    layout, issued at the END of the chain on the ACT HWDGE ring: its ~1.5us
    DMA completion latency is absorbed by the 6-wide round-robin before the
    same individual's next matvec needs y -- no engine FIFO ever waits on a
    DMA completion (that coupling capped earlier versions at ~2.9us/matvec).
"""

import os
import sys
from contextlib import ExitStack

import numpy as np

for _p in ("/opt/trn_rl_repo", "/root/.axon_site/_ro/trn_rl_repo"):
    if os.path.isdir(_p) and _p not in sys.path:
        sys.path.append(_p)

import ml_dtypes  # noqa: E402

import concourse.bass as bass  # noqa: E402
import concourse.tile as tile  # noqa: E402
from concourse import bacc, mybir  # noqa: E402
from concourse.bass_utils import run_bass_kernel_spmd  # noqa: E402

DT = 0.1
ITERS = int(1.0 // DT)  # == 9: 1.0//0.1 == 9.0 in fp
B_FULL, N, OBS, ADIM = 64, 1024, 64, 16
NCORES = 8
BPC = B_FULL // NCORES
P, CN = 128, 8          # n = p*8 + c
F32 = mybir.dt.float32
BF16 = mybir.dt.bfloat16
BF16_NP = ml_dtypes.bfloat16

CG = 4                  # column groups
NSLAB = N // CG         # 256
GSLABS = [(32 * j, 256 * j) for j in range(CG)]  # (tile_col, w_off)
PSW = 512               # one PSUM bank per matvec
PS_BUFS = 8


def wave_schedule(iters=ITERS, mv_us=2.0):
    """Greedy longest-queue-first rounds honoring estimated W arrival.

    Returns rounds (lists of (b, t) work items). Width grows as W tiles land
    (~4.8us each after W0+bulk), then stays ~6-wide so the per-individual
    y chain (incl. scatter DMA latency) hides behind other individuals.
    """
    w_avail = [6.0] + [12.0 + 4.8 * b for b in range(1, BPC)]
    t = 8.0
    remaining = [iters] * BPC
    rounds = []
    while any(remaining):
        active = [b for b in range(BPC) if remaining[b] and w_avail[b] <= t]
        if not active:
            t = min(w_avail[b] for b in range(BPC) if remaining[b])
            continue
        active.sort(key=lambda b: (-remaining[b], b))
        active = active[:6]
        rounds.append([(b, iters - remaining[b]) for b in active])
        for b in active:
            remaining[b] -= 1
        t += max(mv_us * len(active), 8.0 if len(active) < 4 else 0.0)
    return rounds


def make_pools(ctx, tc):
    return dict(
        const=ctx.enter_context(tc.tile_pool(name="const", bufs=1)),
        wpool=ctx.enter_context(tc.tile_pool(name="w", bufs=BPC)),
        et=ctx.enter_context(tc.tile_pool(name="et", bufs=2)),
        row=ctx.enter_context(tc.tile_pool(name="row", bufs=2)),
        tmp=ctx.enter_context(tc.tile_pool(name="tmp", bufs=4)),
        t1p=ctx.enter_context(tc.tile_pool(name="t1p", bufs=8)),
        ps=ctx.enter_context(tc.tile_pool(name="ps", bufs=PS_BUFS, space="PSUM")),
    )


def kernel_body(ctx, tc, ins, out_ap, iters=ITERS, pools=None):
    nc = tc.nc
    Tanh = mybir.ActivationFunctionType.Tanh
    add = mybir.AluOpType.add
    mult = mybir.AluOpType.mult
    sub = mybir.AluOpType.subtract

    p = pools if pools is not None else make_pools(ctx, tc)
    const, wpool, etp, row, tmp, ps = (
        p["const"], p["wpool"], p["et"], p["row"], p["tmp"], p["ps"])
    t1p = p["t1p"]

    # ---- bulk loads on the SP (sync) HWDGE ring: W first ----
    w_sb = []
    for b in range(BPC):
        w_sb.append(wpool.tile([P, CN * N], BF16, tag="w", name=f"w{b}"))
    crow = const.tile([P, BPC * 3 * NSLAB], F32, tag="crow", name="crow")
    nc.sync.dma_start(crow[:], ins["crow"][:])
    bias_c = const.tile([P, BPC * CN], F32, tag="biasc", name="biasc")
    nc.sync.dma_start(bias_c[:], ins["biascol"][:])
    obs_sb = const.tile([OBS, BPC], BF16, tag="obs", name="obs")
    nc.sync.dma_start(obs_sb[:], ins["obsT"][:])
    nc.sync.dma_start(w_sb[0][:], ins["WT"][0])
    vs4 = []
    for b in range(BPC):
        vs4.append(const.tile([P, NSLAB], F32, tag=f"vs{b}", name=f"vs{b}"))
        nc.sync.dma_start(vs4[b][:], ins["vs0row"][:, b * NSLAB:(b + 1) * NSLAB])
    nc.sync.dma_start(w_sb[1][:], ins["WT"][1])
    nc.sync.dma_start(w_sb[2][:], ins["WT"][2])
    dt_sb = const.tile([P, BPC * CN * ADIM], F32, tag="dt", name="dt")
    nc.sync.dma_start(dt_sb[:], ins["DTall"][:])
    for b in range(3, BPC):
        nc.sync.dma_start(w_sb[b][:], ins["WT"][b])

    i4_sb, y_sb = [], []
    for b in range(BPC):
        i4_sb.append(const.tile([P, NSLAB], F32, tag=f"i4{b}", name=f"i4{b}"))
        y_sb.append(const.tile([P, CN], BF16, tag=f"y{b}", name=f"y{b}"))
    act_sb = const.tile([1, BPC * ADIM], F32, tag="act", name="act")

    def cm4_ap(b):
        return crow[:, (b * 3 + 0) * NSLAB:(b * 3 + 1) * NSLAB]

    def g4_ap(b):
        return crow[:, (b * 3 + 1) * NSLAB:(b * 3 + 2) * NSLAB]

    def bc4_ap(b):
        return crow[:, (b * 3 + 2) * NSLAB:(b * 3 + 3) * NSLAB]

    # gate + y scatter: s4 -> tanh -> y4 [128,256] bf16 -> y_col [128,8]
    def emit_y(b, s4):
        y4 = tmp.tile([P, NSLAB], BF16, tag="y4", name="y4")
        nc.scalar.activation(y4[:], s4[:], Tanh)
        nc.scalar.dma_start(y_sb[b][:], y4[0:P:32, :])

    # ---- setup: I row = Ef@obs, scatter to row space, add bc; y0 ----
    for b in range(BPC):
        ir = row.tile([1, N], F32, tag="irow", name=f"ir{b}")
        et = etp.tile([OBS, N], BF16, tag="et", name="et")
        nc.sync.dma_start(et[:], ins["ETall"][:, b * N:(b + 1) * N])
        for h in range(2):
            ip = ps.tile([P, PSW], F32, tag="ps", name=f"ip{b}_{h}")
            nc.tensor.matmul(
                ip[0:1, 0:512],
                obs_sb[:, b:b + 1],
                et[:, h * 512:(h + 1) * 512],
                start=True, stop=True,
            )
            nc.scalar.copy(ir[0:1, h * 512:(h + 1) * 512], ip[0:1, 0:512])
        nc.scalar.dma_start(i4_sb[b][0:P:32, :], ir[:])  # [1,1024]->[4,256] rows
        nc.vector.tensor_tensor(i4_sb[b][:], i4_sb[b][:], bc4_ap(b), op=add)
        s0 = tmp.tile([P, NSLAB], F32, tag="s4", name=f"s0{b}")
        nc.vector.tensor_tensor(s0[:], g4_ap(b), vs4[b][:], op=mult)
        emit_y(b, s0)

    # ---- recurrent loop ----
    t1_sb, wy_sb = {}, {}

    def matvec_mm(b, t):
        # cm*vs only needs last iteration's vs -- runs during the matmuls
        t1 = t1p.tile([P, NSLAB], F32, tag="t1", name="t1")
        nc.vector.tensor_tensor(t1[:], cm4_ap(b), vs4[b][:], op=mult)
        t1_sb[b] = t1
        wy = ps.tile([P, PSW], F32, tag="ps", name="wy")
        wy_sb[b] = wy
        for c in range(CN):
            yc = y_sb[b][:, c:c + 1]
            for (tcol, woff) in GSLABS:
                nc.tensor.matmul(
                    wy[tcol:tcol + 1, 0:NSLAB],
                    yc,
                    w_sb[b][:, c * N + woff: c * N + woff + NSLAB],
                    start=(c == 0), stop=(c == CN - 1),
                    tile_position=(0, tcol),
                )

    def matvec_upd(b, t):
        # row-space leak/gate update straight out of PSUM
        t0 = tmp.tile([P, NSLAB], F32, tag="t0", name="t0")
        nc.vector.tensor_tensor(t0[:], wy_sb[b][:, 0:NSLAB], i4_sb[b][:], op=add)
        nc.vector.tensor_tensor(vs4[b][:], t0[:], t1_sb[b][:], op=add)
        if t < iters - 1:
            s4 = tmp.tile([P, NSLAB], F32, tag="s4", name="s4")
            nc.vector.tensor_tensor(s4[:], g4_ap(b), vs4[b][:], op=mult)
            emit_y(b, s4)

    # ---- decode: action = D @ (vs - bias) ----
    vcol_sb = {}

    def decode_pre(b):
        # issued right after b's last update; completes long before the
        # end-of-kernel decode matmuls so they never stall PE
        vcol = const.tile([P, CN], F32, tag=f"vc{b}", name=f"vc{b}")
        nc.scalar.dma_start(vcol[:], vs4[b][0:P:32, :])
        vcol_sb[b] = vcol

    def decode(b):
        vf = tmp.tile([P, CN], F32, tag="vf", name="vf")
        nc.vector.tensor_tensor(
            vf[:], vcol_sb[b][:], bias_c[:, b * CN:(b + 1) * CN], op=sub)
        ap = ps.tile([P, PSW], F32, tag="ps", name="dec")
        for c in range(CN):
            nc.tensor.matmul(
                ap[0:1, 0:ADIM],
                vf[:, c:c + 1],
                dt_sb[:, b * CN * ADIM + c * ADIM: b * CN * ADIM + (c + 1) * ADIM],
                start=(c == 0), stop=(c == CN - 1),
            )
        nc.vector.tensor_copy(act_sb[0:1, b * ADIM:(b + 1) * ADIM], ap[0:1, 0:ADIM])

    for round_items in wave_schedule(iters):
        for b, t in round_items:
            matvec_mm(b, t)
        for b, t in round_items:
            matvec_upd(b, t)
            if t == iters - 1:
                decode_pre(b)
    for b in range(BPC):
        decode(b)
    nc.sync.dma_start(out_ap[:], act_sb[0:1, :])


def build_nc(iters=ITERS):
    nc = bacc.Bacc(
        "TRN2", target_bir_lowering=False, debug=False, enable_asserts=False,
    )
    ins = {}
    ins["WT"] = nc.dram_tensor("WT", [BPC, P, CN * N], BF16, kind="ExternalInput").ap()
    ins["ETall"] = nc.dram_tensor("ETall", [OBS, BPC * N], BF16, kind="ExternalInput").ap()
    ins["obsT"] = nc.dram_tensor("obsT", [OBS, BPC], BF16, kind="ExternalInput").ap()
    ins["crow"] = nc.dram_tensor(
        "crow", [P, BPC * 3 * NSLAB], F32, kind="ExternalInput").ap()
    ins["vs0row"] = nc.dram_tensor(
        "vs0row", [P, BPC * NSLAB], F32, kind="ExternalInput").ap()
    ins["biascol"] = nc.dram_tensor(
        "biascol", [P, BPC * CN], F32, kind="ExternalInput").ap()
    ins["DTall"] = nc.dram_tensor(
        "DTall", [P, BPC * CN * ADIM], F32, kind="ExternalInput").ap()
    out_ap = nc.dram_tensor("act", [BPC, ADIM], F32, kind="ExternalOutput").ap()

    with tile.TileContext(nc) as tc:
        with ExitStack() as ctx:
            pools = make_pools(ctx, tc)
            kernel_body(ctx, tc, ins, out_ap, iters, pools)
    nc.compile()
    return nc


def _to_rowspace(arr):
    """[B, N] -> [B, 128, NSLAB] row-space: row 32j holds n-slab j, rest 0."""
    B = arr.shape[0]
    out = np.zeros((B, P, NSLAB), np.float32)
    for j in range(CG):
        out[:, 32 * j, :] = arr[:, NSLAB * j:NSLAB * (j + 1)]
    return out


def prep_in_maps(obs, v0, tau, gain, bias, W, mask, E, D):
    f = np.float32
    obs, v0, tau, gain, bias, W, mask, E, D = [
        np.asarray(x, dtype=f) for x in (obs, v0, tau, gain, bias, W, mask, E, D)
    ]
    am = (DT / tau) * mask                    # [64, N]
    cm = (1.0 - DT / tau) * mask
    Wf = W * am[:, :, None] * mask[:, None, :]
    WT = np.ascontiguousarray(Wf.transpose(0, 2, 1)).reshape(
        B_FULL, P, CN * N).astype(BF16_NP)
    ETp = np.ascontiguousarray(
        (E * am[:, :, None]).transpose(0, 2, 1)).astype(BF16_NP)  # [64, OBS, N]
    DTp = np.ascontiguousarray(D.transpose(0, 2, 1)).reshape(B_FULL, P, CN * ADIM)
    obsT = np.ascontiguousarray(obs.T).astype(BF16_NP)  # [OBS, 64]
    cm4 = _to_rowspace(cm)
    g4 = _to_rowspace(gain)
    bc4 = _to_rowspace(bias * (1.0 - cm))
    vs04 = _to_rowspace(v0 + bias)
    biascol = bias.reshape(B_FULL, P, CN)

    in_maps = []
    for core in range(NCORES):
        s = slice(core * BPC, (core + 1) * BPC)
        crow = np.empty((P, BPC * 3 * NSLAB), f)
        for i, b in enumerate(range(core * BPC, (core + 1) * BPC)):
            for k, arr in enumerate((cm4, g4, bc4)):
                crow[:, (i * 3 + k) * NSLAB:(i * 3 + k + 1) * NSLAB] = arr[b]
        vs0row = np.ascontiguousarray(
            vs04[s].transpose(1, 0, 2).reshape(P, BPC * NSLAB))
        bcol = np.ascontiguousarray(
            biascol[s].transpose(1, 0, 2).reshape(P, BPC * CN))
        et = np.ascontiguousarray(
            ETp[s].transpose(1, 0, 2).reshape(OBS, BPC * N))
        dtall = np.ascontiguousarray(
            DTp[s].transpose(1, 0, 2).reshape(P, BPC * CN * ADIM))
        in_maps.append({
            "WT": np.ascontiguousarray(WT[s]),
            "ETall": et,
            "obsT": np.ascontiguousarray(obsT[:, s]),
            "crow": crow,
            "vs0row": vs0row,
            "biascol": bcol,
            "DTall": dtall,
        })
    return in_maps


_NC_CACHE = None


def _get_nc():
    global _NC_CACHE
    if _NC_CACHE is None:
        _NC_CACHE = build_nc()
    return _NC_CACHE


def kernel(obs, v0, tau, gain, bias, W, mask, E, D):
    nc = _get_nc()
    in_maps = prep_in_maps(obs, v0, tau, gain, bias, W, mask, E, D)
    res = run_bass_kernel_spmd(nc, in_maps, core_ids=list(range(NCORES)))
    return np.concatenate([res.results[c]["act"] for c in range(NCORES)], axis=0)


# revision 26
# speedup vs baseline: 1.3600x; 1.0978x over previous
"""CTRNN policy kernel for Trainium2 (8 NeuronCores, batch-parallel).

Reference computation (per batch element b, B=64, N=1024, OBS=64, A=16):
    I = E[b] @ obs[b]
    repeat int(1.0//0.1)=9 times:
        y = tanh(gain*(v+bias))*mask
        v = (v + DT/tau * (-v + W[b]@y + I)) * mask
    action[b] = D[b] @ v

Sharding: batch 64 -> 8 cores x 8 individuals, fully data parallel.

Per-core algebra (host-folded, mask/tau folded into the coefficients):
    am = DT/tau*mask, cm = (1-DT/tau)*mask
    Wf = diag(am) @ W @ diag(mask);  Ef = diag(am) @ E;  bc = bias*(1-cm)
    state vs = v + bias:
        y   = tanh(g * vs)
        vs' = cm*vs + Wf@y + (Ef@obs + bc)
    action = D @ (vs - bias)

Device mapping per individual (N=1024 as n = p*8 + c for the matmul
contraction; W^T slabs [128, 8192] bf16 all resident in SBUF):

  - matvec on TensorE with 4-way column-group tiling: stationary = y column
    chunk [128,1] bf16 at array column 32j, moving = W^T n-slab [128,256].
    The 4 groups stream concurrently (separate XBUSes) and land in ONE
    shared PSUM bank at partitions {0,32,64,96} (disjoint per-partition
    accumulators), so a matvec costs ~1.9us of PE instead of ~3.5us.
  - the leak/gate update runs in "row space" [128,256] right out of PSUM
    (rows 32j hold dv n-slab j; other lanes carry zeros): tensor_tensor ops
    are lane-parallel so the garbage lanes are free. The only partition
    redistribution is the y scatter [4x256 rows] -> [128,8] bf16 column
    layout, issued at the END of the chain on the ACT HWDGE ring: its ~1.5us
    DMA completion latency is absorbed by the 6-wide round-robin before the
    same individual's next matvec needs y -- no engine FIFO ever waits on a
    DMA completion (that coupling capped earlier versions at ~2.9us/matvec).
"""

import os
import sys
from contextlib import ExitStack

import numpy as np

for _p in ("/opt/trn_rl_repo", "/root/.axon_site/_ro/trn_rl_repo"):
    if os.path.isdir(_p) and _p not in sys.path:
        sys.path.append(_p)

import ml_dtypes  # noqa: E402

import concourse.bass as bass  # noqa: E402
import concourse.tile as tile  # noqa: E402
from concourse import bacc, mybir  # noqa: E402
from concourse.bass_utils import run_bass_kernel_spmd  # noqa: E402

DT = 0.1
ITERS = int(1.0 // DT)  # == 9: 1.0//0.1 == 9.0 in fp
B_FULL, N, OBS, ADIM = 64, 1024, 64, 16
NCORES = 8
BPC = B_FULL // NCORES
P, CN = 128, 8          # n = p*8 + c
F32 = mybir.dt.float32
BF16 = mybir.dt.bfloat16
BF16_NP = ml_dtypes.bfloat16

CG = 4                  # column groups
NSLAB = N // CG         # 256
GSLABS = [(32 * j, 256 * j) for j in range(CG)]  # (tile_col, w_off)
PSW = 512               # one PSUM bank per matvec
PS_BUFS = 8


def wave_schedule(iters=ITERS, mv_us=2.0):
    """Greedy longest-queue-first rounds honoring estimated W arrival.

    Returns rounds (lists of (b, t) work items). Width grows as W tiles land
    (~4.8us each after W0+bulk), then stays ~6-wide so the per-individual
    y chain (incl. scatter DMA latency) hides behind other individuals.
    """
    w_avail = [7.0, 20.0, 27.0, 33.0, 38.0, 43.0, 48.0, 53.0]
    t = 16.0
    remaining = [iters] * BPC
    rounds = []
    while any(remaining):
        active = [b for b in range(BPC) if remaining[b] and w_avail[b] <= t]
        if not active:
            t = min(w_avail[b] for b in range(BPC) if remaining[b])
            continue
        active.sort(key=lambda b: (-remaining[b], b))
        active = active[:6]
        rounds.append([(b, iters - remaining[b]) for b in active])
        for b in active:
            remaining[b] -= 1
        t += max(mv_us * len(active), 8.0 if len(active) < 4 else 0.0)
    return rounds


def make_pools(ctx, tc):
    return dict(
        const=ctx.enter_context(tc.tile_pool(name="const", bufs=1)),
        wpool=ctx.enter_context(tc.tile_pool(name="w", bufs=BPC)),
        et=ctx.enter_context(tc.tile_pool(name="et", bufs=2)),
        row=ctx.enter_context(tc.tile_pool(name="row", bufs=2)),
        tmp=ctx.enter_context(tc.tile_pool(name="tmp", bufs=4)),
        t1p=ctx.enter_context(tc.tile_pool(name="t1p", bufs=8)),
        ps=ctx.enter_context(tc.tile_pool(name="ps", bufs=PS_BUFS, space="PSUM")),
    )


def kernel_body(ctx, tc, ins, out_ap, iters=ITERS, pools=None):
    nc = tc.nc
    Tanh = mybir.ActivationFunctionType.Tanh
    add = mybir.AluOpType.add
    mult = mybir.AluOpType.mult
    sub = mybir.AluOpType.subtract

    p = pools if pools is not None else make_pools(ctx, tc)
    const, wpool, etp, row, tmp, ps = (
        p["const"], p["wpool"], p["et"], p["row"], p["tmp"], p["ps"])
    t1p = p["t1p"]

    # ---- bulk loads on the SP (sync) HWDGE ring: W first ----
    w_sb = []
    for b in range(BPC):
        w_sb.append(wpool.tile([P, CN * N], BF16, tag="w", name=f"w{b}"))
    nc.sync.dma_start(w_sb[0][:], ins["WT"][0])
    crow = const.tile([P, BPC * 3 * NSLAB], F32, tag="crow", name="crow")
    nc.sync.dma_start(crow[:], ins["crow"][:])
    bias_c = const.tile([P, BPC * CN], F32, tag="biasc", name="biasc")
    nc.sync.dma_start(bias_c[:], ins["biascol"][:])
    obs_sb = const.tile([OBS, BPC], BF16, tag="obs", name="obs")
    nc.sync.dma_start(obs_sb[:], ins["obsT"][:])
    nc.sync.dma_start(w_sb[1][:], ins["WT"][1])
    vs4 = []
    for b in range(BPC):
        vs4.append(const.tile([P, NSLAB], F32, tag=f"vs{b}", name=f"vs{b}"))
        nc.sync.dma_start(vs4[b][:], ins["vs0row"][:, b * NSLAB:(b + 1) * NSLAB])
    nc.sync.dma_start(w_sb[2][:], ins["WT"][2])
    dt_sb = const.tile([P, BPC * CN * ADIM], F32, tag="dt", name="dt")
    nc.sync.dma_start(dt_sb[:], ins["DTall"][:])
    for b in range(3, BPC):
        nc.sync.dma_start(w_sb[b][:], ins["WT"][b])

    i4_sb, y_sb = [], []
    for b in range(BPC):
        i4_sb.append(const.tile([P, NSLAB], F32, tag=f"i4{b}", name=f"i4{b}"))
        y_sb.append(const.tile([P, CN], BF16, tag=f"y{b}", name=f"y{b}"))
    act_sb = const.tile([1, BPC * ADIM], F32, tag="act", name="act")

    def cm4_ap(b):
        return crow[:, (b * 3 + 0) * NSLAB:(b * 3 + 1) * NSLAB]

    def g4_ap(b):
        return crow[:, (b * 3 + 1) * NSLAB:(b * 3 + 2) * NSLAB]

    def bc4_ap(b):
        return crow[:, (b * 3 + 2) * NSLAB:(b * 3 + 3) * NSLAB]

    # gate + y scatter: s4 -> tanh -> y4 [128,256] bf16 -> y_col [128,8]
    def emit_y(b, s4):
        y4 = tmp.tile([P, NSLAB], BF16, tag="y4", name="y4")
        nc.scalar.activation(y4[:], s4[:], Tanh)
        nc.scalar.dma_start(y_sb[b][:], y4[0:P:32, :])

    # ---- setup: I row = Ef@obs, scatter to row space, add bc; y0 ----
    for b in range(BPC):
        ir = row.tile([1, N], F32, tag="irow", name=f"ir{b}")
        et = etp.tile([OBS, N], BF16, tag="et", name="et")
        # ACT-ring DMA: must not queue behind the W stream on the SP ring
        nc.scalar.dma_start(et[:], ins["ETall"][:, b * N:(b + 1) * N])
        for h in range(2):
            ip = ps.tile([P, PSW], F32, tag="ps", name=f"ip{b}_{h}")
            nc.tensor.matmul(
                ip[0:1, 0:512],
                obs_sb[:, b:b + 1],
                et[:, h * 512:(h + 1) * 512],
                start=True, stop=True,
            )
            nc.scalar.copy(ir[0:1, h * 512:(h + 1) * 512], ip[0:1, 0:512])
        nc.scalar.dma_start(i4_sb[b][0:P:32, :], ir[:])  # [1,1024]->[4,256] rows
        nc.vector.tensor_tensor(i4_sb[b][:], i4_sb[b][:], bc4_ap(b), op=add)
        s0 = tmp.tile([P, NSLAB], F32, tag="s4", name=f"s0{b}")
        nc.vector.tensor_tensor(s0[:], g4_ap(b), vs4[b][:], op=mult)
        emit_y(b, s0)

    # ---- recurrent loop ----
    t1_sb, wy_sb = {}, {}

    def matvec_mm(b, t):
        # cm*vs only needs last iteration's vs -- runs during the matmuls
        t1 = t1p.tile([P, NSLAB], F32, tag="t1", name="t1")
        nc.vector.tensor_tensor(t1[:], cm4_ap(b), vs4[b][:], op=mult)
        t1_sb[b] = t1
        wy = ps.tile([P, PSW], F32, tag="ps", name="wy")
        wy_sb[b] = wy
        for c in range(CN):
            yc = y_sb[b][:, c:c + 1]
            for (tcol, woff) in GSLABS:
                nc.tensor.matmul(
                    wy[tcol:tcol + 1, 0:NSLAB],
                    yc,
                    w_sb[b][:, c * N + woff: c * N + woff + NSLAB],
                    start=(c == 0), stop=(c == CN - 1),
                    tile_position=(0, tcol),
                )

    def matvec_upd(b, t):
        # row-space leak/gate update straight out of PSUM
        t0 = tmp.tile([P, NSLAB], F32, tag="t0", name="t0")
        nc.vector.tensor_tensor(t0[:], wy_sb[b][:, 0:NSLAB], i4_sb[b][:], op=add)
        nc.vector.tensor_tensor(vs4[b][:], t0[:], t1_sb[b][:], op=add)
        if t < iters - 1:
            s4 = tmp.tile([P, NSLAB], F32, tag="s4", name="s4")
            nc.vector.tensor_tensor(s4[:], g4_ap(b), vs4[b][:], op=mult)
            emit_y(b, s4)

    # ---- decode: action = D @ (vs - bias) ----
    vcol_sb = {}

    def decode_pre(b):
        # issued right after b's last update; completes long before the
        # end-of-kernel decode matmuls so they never stall PE
        vcol = const.tile([P, CN], F32, tag=f"vc{b}", name=f"vc{b}")
        nc.scalar.dma_start(vcol[:], vs4[b][0:P:32, :])
        vcol_sb[b] = vcol

    def decode(b):
        vf = tmp.tile([P, CN], F32, tag="vf", name="vf")
        nc.vector.tensor_tensor(
            vf[:], vcol_sb[b][:], bias_c[:, b * CN:(b + 1) * CN], op=sub)
        ap = ps.tile([P, PSW], F32, tag="ps", name="dec")
        for c in range(CN):
            nc.tensor.matmul(
                ap[0:1, 0:ADIM],
                vf[:, c:c + 1],
                dt_sb[:, b * CN * ADIM + c * ADIM: b * CN * ADIM + (c + 1) * ADIM],
                start=(c == 0), stop=(c == CN - 1),
            )
        nc.vector.tensor_copy(act_sb[0:1, b * ADIM:(b + 1) * ADIM], ap[0:1, 0:ADIM])

    for round_items in wave_schedule(iters):
        for b, t in round_items:
            matvec_mm(b, t)
        for b, t in round_items:
            matvec_upd(b, t)
            if t == iters - 1:
                decode_pre(b)
    for b in range(BPC):
        decode(b)
    nc.sync.dma_start(out_ap[:], act_sb[0:1, :])


def build_nc(iters=ITERS):
    nc = bacc.Bacc(
        "TRN2", target_bir_lowering=False, debug=False, enable_asserts=False,
    )
    ins = {}
    ins["WT"] = nc.dram_tensor("WT", [BPC, P, CN * N], BF16, kind="ExternalInput").ap()
    ins["ETall"] = nc.dram_tensor("ETall", [OBS, BPC * N], BF16, kind="ExternalInput").ap()
    ins["obsT"] = nc.dram_tensor("obsT", [OBS, BPC], BF16, kind="ExternalInput").ap()
    ins["crow"] = nc.dram_tensor(
        "crow", [P, BPC * 3 * NSLAB], F32, kind="ExternalInput").ap()
    ins["vs0row"] = nc.dram_tensor(
        "vs0row", [P, BPC * NSLAB], F32, kind="ExternalInput").ap()
    ins["biascol"] = nc.dram_tensor(
        "biascol", [P, BPC * CN], F32, kind="ExternalInput").ap()
    ins["DTall"] = nc.dram_tensor(
        "DTall", [P, BPC * CN * ADIM], F32, kind="ExternalInput").ap()
    out_ap = nc.dram_tensor("act", [BPC, ADIM], F32, kind="ExternalOutput").ap()

    with tile.TileContext(nc) as tc:
        with ExitStack() as ctx:
            pools = make_pools(ctx, tc)
            kernel_body(ctx, tc, ins, out_ap, iters, pools)
    nc.compile()
    return nc


def _to_rowspace(arr):
    """[B, N] -> [B, 128, NSLAB] row-space: row 32j holds n-slab j, rest 0."""
    B = arr.shape[0]
    out = np.zeros((B, P, NSLAB), np.float32)
    for j in range(CG):
        out[:, 32 * j, :] = arr[:, NSLAB * j:NSLAB * (j + 1)]
    return out


def prep_in_maps(obs, v0, tau, gain, bias, W, mask, E, D):
    f = np.float32
    obs, v0, tau, gain, bias, W, mask, E, D = [
        np.asarray(x, dtype=f) for x in (obs, v0, tau, gain, bias, W, mask, E, D)
    ]
    am = (DT / tau) * mask                    # [64, N]
    cm = (1.0 - DT / tau) * mask
    Wf = W * am[:, :, None] * mask[:, None, :]
    WT = np.ascontiguousarray(Wf.transpose(0, 2, 1)).reshape(
        B_FULL, P, CN * N).astype(BF16_NP)
    ETp = np.ascontiguousarray(
        (E * am[:, :, None]).transpose(0, 2, 1)).astype(BF16_NP)  # [64, OBS, N]
    DTp = np.ascontiguousarray(D.transpose(0, 2, 1)).reshape(B_FULL, P, CN * ADIM)
    obsT = np.ascontiguousarray(obs.T).astype(BF16_NP)  # [OBS, 64]
    cm4 = _to_rowspace(cm)
    g4 = _to_rowspace(gain)
    bc4 = _to_rowspace(bias * (1.0 - cm))
    vs04 = _to_rowspace(v0 + bias)
    biascol = bias.reshape(B_FULL, P, CN)

    in_maps = []
    for core in range(NCORES):
        s = slice(core * BPC, (core + 1) * BPC)
        crow = np.empty((P, BPC * 3 * NSLAB), f)
        for i, b in enumerate(range(core * BPC, (core + 1) * BPC)):
            for k, arr in enumerate((cm4, g4, bc4)):
                crow[:, (i * 3 + k) * NSLAB:(i * 3 + k + 1) * NSLAB] = arr[b]
        vs0row = np.ascontiguousarray(
            vs04[s].transpose(1, 0, 2).reshape(P, BPC * NSLAB))
        bcol = np.ascontiguousarray(
            biascol[s].transpose(1, 0, 2).reshape(P, BPC * CN))
        et = np.ascontiguousarray(
            ETp[s].transpose(1, 0, 2).reshape(OBS, BPC * N))
        dtall = np.ascontiguousarray(
            DTp[s].transpose(1, 0, 2).reshape(P, BPC * CN * ADIM))
        in_maps.append({
            "WT": np.ascontiguousarray(WT[s]),
            "ETall": et,
            "obsT": np.ascontiguousarray(obsT[:, s]),
            "crow": crow,
            "vs0row": vs0row,
            "biascol": bcol,
            "DTall": dtall,
        })
    return in_maps


_NC_CACHE = None


def _get_nc():
    global _NC_CACHE
    if _NC_CACHE is None:
        _NC_CACHE = build_nc()
    return _NC_CACHE


def kernel(obs, v0, tau, gain, bias, W, mask, E, D):
    nc = _get_nc()
    in_maps = prep_in_maps(obs, v0, tau, gain, bias, W, mask, E, D)
    res = run_bass_kernel_spmd(nc, in_maps, core_ids=list(range(NCORES)))
    return np.concatenate([res.results[c]["act"] for c in range(NCORES)], axis=0)


# revision 37
# speedup vs baseline: 1.4081x; 1.0353x over previous
"""CTRNN policy kernel for Trainium2 (8 NeuronCores, batch-parallel).

Reference computation (per batch element b, B=64, N=1024, OBS=64, A=16):
    I = E[b] @ obs[b]
    repeat int(1.0//0.1)=9 times:
        y = tanh(gain*(v+bias))*mask
        v = (v + DT/tau * (-v + W[b]@y + I)) * mask
    action[b] = D[b] @ v

Sharding: batch 64 -> 8 cores x 8 individuals, fully data parallel.

Per-core algebra (host-folded, mask/tau folded into the coefficients):
    am = DT/tau*mask, cm = (1-DT/tau)*mask
    Wf = diag(am) @ W @ diag(mask);  Ef = diag(am) @ E;  bc = bias*(1-cm)
    state vs = v + bias:
        y   = tanh(g * vs)
        vs' = cm*vs + Wf@y + (Ef@obs + bc)
    action = D @ (vs - bias)

Device mapping per individual (N=1024 as n = p*8 + c for the matmul
contraction; W^T slabs [128, 8192] bf16 all resident in SBUF):

  - matvec on TensorE with 4-way column-group tiling: stationary = y column
    chunk [128,1] bf16 at array column 32j, moving = W^T n-slab [128,256].
    The 4 groups stream concurrently (separate XBUSes) and land in ONE
    shared PSUM bank at partitions {0,32,64,96} (disjoint per-partition
    accumulators), so a matvec costs ~1.9us of PE instead of ~3.5us.
  - the leak/gate update runs in "row space" [128,256] right out of PSUM
    (rows 32j hold dv n-slab j; other lanes carry zeros): tensor_tensor ops
    are lane-parallel so the garbage lanes are free. The only partition
    redistribution is the y scatter [4x256 rows] -> [128,8] bf16 column
    layout, issued at the END of the chain on the ACT HWDGE ring: its ~1.5us
    DMA completion latency is absorbed by the 6-wide round-robin before the
    same individual's next matvec needs y -- no engine FIFO ever waits on a
    DMA completion (that coupling capped earlier versions at ~2.9us/matvec).
"""

import os
import sys
from contextlib import ExitStack

import numpy as np

for _p in ("/opt/trn_rl_repo", "/root/.axon_site/_ro/trn_rl_repo"):
    if os.path.isdir(_p) and _p not in sys.path:
        sys.path.append(_p)

import ml_dtypes  # noqa: E402

import concourse.bass as bass  # noqa: E402
import concourse.tile as tile  # noqa: E402
from concourse import bacc, mybir  # noqa: E402
from concourse.bass_utils import run_bass_kernel_spmd  # noqa: E402

DT = 0.1
ITERS = int(1.0 // DT)  # == 9: 1.0//0.1 == 9.0 in fp
B_FULL, N, OBS, ADIM = 64, 1024, 64, 16
NCORES = 8
BPC = B_FULL // NCORES
P, CN = 128, 8          # n = p*8 + c
F32 = mybir.dt.float32
BF16 = mybir.dt.bfloat16
BF16_NP = ml_dtypes.bfloat16

CG = 4                  # column groups
NSLAB = N // CG         # 256
GSLABS = [(32 * j, 256 * j) for j in range(CG)]  # (tile_col, w_off)
PSW = 512               # one PSUM bank per matvec
PS_BUFS = 8

# contraction chunk map for the DVE-block-transpose y distribution:
# chunk k's stationary column is yT[:, 32k] where yT = 32x32-block-transpose
# of row-space y4 (rows 32j hold n-slab j). That column holds
# y[m_k(p)] with m_k(p) = 256*(p//32) + 32*k + (p%32); the host permutes the
# W^T slabs to match, so no DMA scatter is needed anywhere in the loop.
_pidx = np.arange(P)
M_INDEX = (256 * (_pidx[:, None] // 32) + 32 * np.arange(CN)[None, :]
           + (_pidx[:, None] % 32))  # [128, 8]


def wave_schedule(iters=ITERS, mv_us=2.0):
    """Greedy longest-queue-first rounds honoring estimated W arrival.

    Returns rounds (lists of (b, t) work items). Width grows as W tiles land
    (~4.8us each after W0+bulk), then stays ~6-wide so the per-individual
    y chain (incl. scatter DMA latency) hides behind other individuals.
    """
    w_avail = [9.5, 14.5, 21.0, 26.5, 31.5, 36.5, 41.5, 46.5]
    t = 10.0
    remaining = [iters] * BPC
    rounds = []
    while any(remaining):
        active = [b for b in range(BPC) if remaining[b] and w_avail[b] <= t]
        if not active:
            t = min(w_avail[b] for b in range(BPC) if remaining[b])
            continue
        active.sort(key=lambda b: (-remaining[b], b))
        active = active[:6]
        rounds.append([(b, iters - remaining[b]) for b in active])
        for b in active:
            remaining[b] -= 1
        t += max(mv_us * len(active), 8.0 if len(active) < 4 else 0.0)
    return rounds


def make_pools(ctx, tc):
    return dict(
        const=ctx.enter_context(tc.tile_pool(name="const", bufs=1)),
        wpool=ctx.enter_context(tc.tile_pool(name="w", bufs=BPC)),
        et=ctx.enter_context(tc.tile_pool(name="et", bufs=2)),
        row=ctx.enter_context(tc.tile_pool(name="row", bufs=2)),
        tmp=ctx.enter_context(tc.tile_pool(name="tmp", bufs=4)),
        t1p=ctx.enter_context(tc.tile_pool(name="t1p", bufs=8)),
        ps=ctx.enter_context(tc.tile_pool(name="ps", bufs=PS_BUFS, space="PSUM")),
    )


def kernel_body(ctx, tc, ins, out_ap, iters=ITERS, pools=None, cm_const=None):
    nc = tc.nc
    Tanh = mybir.ActivationFunctionType.Tanh
    add = mybir.AluOpType.add
    mult = mybir.AluOpType.mult
    sub = mybir.AluOpType.subtract

    p = pools if pools is not None else make_pools(ctx, tc)
    const, wpool, etp, row, tmp, ps = (
        p["const"], p["wpool"], p["et"], p["row"], p["tmp"], p["ps"])
    t1p = p["t1p"]

    # ---- bulk loads on the SP (sync) HWDGE ring: W first ----
    w_sb = []
    for b in range(BPC):
        w_sb.append(wpool.tile([P, CN * N], BF16, tag="w", name=f"w{b}"))
    crow = const.tile([P, BPC * 3 * NSLAB], F32, tag="crow", name="crow")
    nc.sync.dma_start(crow[:], ins["crow"][:])
    bias_c = const.tile([P, BPC * CN], F32, tag="biasc", name="biasc")
    nc.sync.dma_start(bias_c[:], ins["biascol"][:])
    obs_sb = const.tile([OBS, BPC], BF16, tag="obs", name="obs")
    nc.sync.dma_start(obs_sb[:], ins["obsT"][:])
    nc.sync.dma_start(w_sb[0][:], ins["WT"][0])
    nc.sync.dma_start(w_sb[1][:], ins["WT"][1])
    vs4 = []
    for b in range(BPC):
        vs4.append(const.tile([P, NSLAB], F32, tag=f"vs{b}", name=f"vs{b}"))
        nc.sync.dma_start(vs4[b][:], ins["vs0row"][:, b * NSLAB:(b + 1) * NSLAB])
    nc.sync.dma_start(w_sb[2][:], ins["WT"][2])
    dt_sb = const.tile([P, BPC * CN * ADIM], F32, tag="dt", name="dt")
    nc.sync.dma_start(dt_sb[:], ins["DTall"][:])
    for b in range(3, BPC):
        nc.sync.dma_start(w_sb[b][:], ins["WT"][b])

    i4_sb, y_sb = [], []
    for b in range(BPC):
        i4_sb.append(const.tile([P, NSLAB], F32, tag=f"i4{b}", name=f"i4{b}"))
        y_sb.append(const.tile([P, NSLAB], BF16, tag=f"y{b}", name=f"y{b}"))
    act_sb = const.tile([1, BPC * ADIM], F32, tag="act", name="act")

    def cm4_ap(b):
        return crow[:, (b * 3 + 0) * NSLAB:(b * 3 + 1) * NSLAB]

    def g4_ap(b):
        return crow[:, (b * 3 + 1) * NSLAB:(b * 3 + 2) * NSLAB]

    def bc4_ap(b):
        return crow[:, (b * 3 + 2) * NSLAB:(b * 3 + 3) * NSLAB]

    # gate + y distribution: s4 -> tanh -> y4 [128,256] bf16 -> DVE 32x32
    # block-transpose: yT[:, 32k] is chunk k's stationary column. No DMA.
    def emit_y(b, s4):
        y4 = tmp.tile([P, NSLAB], BF16, tag="y4", name="y4")
        nc.scalar.activation(y4[:], s4[:], Tanh)
        nc.vector.transpose(y_sb[b][:], y4[:])

    # ---- setup: I row = Ef@obs, scatter to row space, add bc; y0 ----
    for b in range(BPC):
        ir = row.tile([1, N], F32, tag="irow", name=f"ir{b}")
        et = etp.tile([OBS, N], BF16, tag="et", name="et")
        # ACT-ring DMA: must not queue behind the W stream on the SP ring
        nc.scalar.dma_start(et[:], ins["ETall"][:, b * N:(b + 1) * N])
        for h in range(2):
            ip = ps.tile([P, PSW], F32, tag="ps", name=f"ip{b}_{h}")
            nc.tensor.matmul(
                ip[0:1, 0:512],
                obs_sb[:, b:b + 1],
                et[:, h * 512:(h + 1) * 512],
                start=True, stop=True,
            )
            nc.scalar.copy(ir[0:1, h * 512:(h + 1) * 512], ip[0:1, 0:512])
        nc.scalar.dma_start(i4_sb[b][0:P:32, :], ir[:])  # [1,1024]->[4,256] rows
        nc.vector.tensor_tensor(i4_sb[b][:], i4_sb[b][:], bc4_ap(b), op=add)
        s0 = tmp.tile([P, NSLAB], F32, tag="s4", name=f"s0{b}")
        nc.vector.tensor_tensor(s0[:], g4_ap(b), vs4[b][:], op=mult)
        emit_y(b, s0)

    # ---- recurrent loop ----
    t1_sb, wy_sb = {}, {}

    def matvec_mm(b, t):
        # t1 = cm*vs + I only needs last iteration's vs -- runs during the
        # matmuls, off the post-matmul chain. With constant tau/mask the
        # leak multiply is a ScalarE const-mul, keeping VectorE under PE.
        tg = tmp.tile([P, NSLAB], F32, tag="tg", name="tg")
        if cm_const is not None:
            nc.scalar.mul(tg[:], vs4[b][:], cm_const)
        else:
            nc.vector.tensor_tensor(tg[:], cm4_ap(b), vs4[b][:], op=mult)
        t1 = t1p.tile([P, NSLAB], F32, tag="t1", name="t1")
        nc.vector.tensor_tensor(t1[:], tg[:], i4_sb[b][:], op=add)
        t1_sb[b] = t1
        wy = ps.tile([P, PSW], F32, tag="ps", name="wy")
        wy_sb[b] = wy
        for c in range(CN):
            yc = y_sb[b][:, 32 * c:32 * c + 1]
            for (tcol, woff) in GSLABS:
                nc.tensor.matmul(
                    wy[tcol:tcol + 1, 0:NSLAB],
                    yc,
                    w_sb[b][:, c * N + woff: c * N + woff + NSLAB],
                    start=(c == 0), stop=(c == CN - 1),
                    tile_position=(0, tcol),
                )

    def matvec_upd(b, t):
        # row-space leak/gate update straight out of PSUM
        nc.vector.tensor_tensor(vs4[b][:], wy_sb[b][:, 0:NSLAB], t1_sb[b][:],
                                op=add)
        if t < iters - 1:
            s4 = tmp.tile([P, NSLAB], F32, tag="s4", name="s4")
            nc.vector.tensor_tensor(s4[:], g4_ap(b), vs4[b][:], op=mult)
            emit_y(b, s4)

    # ---- decode: action = D @ (vs - bias) ----
    vcol_sb = {}

    def decode_pre(b):
        # issued right after b's last update; completes long before the
        # end-of-kernel decode matmuls so they never stall PE
        vcol = const.tile([P, CN], F32, tag=f"vc{b}", name=f"vc{b}")
        nc.scalar.dma_start(vcol[:], vs4[b][0:P:32, :])
        vcol_sb[b] = vcol

    def decode(b):
        vf = tmp.tile([P, CN], F32, tag="vf", name="vf")
        nc.vector.tensor_tensor(
            vf[:], vcol_sb[b][:], bias_c[:, b * CN:(b + 1) * CN], op=sub)
        ap = ps.tile([P, PSW], F32, tag="ps", name="dec")
        for c in range(CN):
            nc.tensor.matmul(
                ap[0:1, 0:ADIM],
                vf[:, c:c + 1],
                dt_sb[:, b * CN * ADIM + c * ADIM: b * CN * ADIM + (c + 1) * ADIM],
                start=(c == 0), stop=(c == CN - 1),
            )
        nc.vector.tensor_copy(act_sb[0:1, b * ADIM:(b + 1) * ADIM], ap[0:1, 0:ADIM])

    for round_items in wave_schedule(iters):
        for b, t in round_items:
            matvec_mm(b, t)
        for b, t in round_items:
            matvec_upd(b, t)
            if t == iters - 1:
                decode_pre(b)
    for b in range(BPC):
        decode(b)
    nc.sync.dma_start(out_ap[:], act_sb[0:1, :])


def build_nc(iters=ITERS, cm_const=None):
    nc = bacc.Bacc(
        "TRN2", target_bir_lowering=False, debug=False, enable_asserts=False,
    )
    ins = {}
    ins["WT"] = nc.dram_tensor("WT", [BPC, P, CN * N], BF16, kind="ExternalInput").ap()
    ins["ETall"] = nc.dram_tensor("ETall", [OBS, BPC * N], BF16, kind="ExternalInput").ap()
    ins["obsT"] = nc.dram_tensor("obsT", [OBS, BPC], BF16, kind="ExternalInput").ap()
    ins["crow"] = nc.dram_tensor(
        "crow", [P, BPC * 3 * NSLAB], F32, kind="ExternalInput").ap()
    ins["vs0row"] = nc.dram_tensor(
        "vs0row", [P, BPC * NSLAB], F32, kind="ExternalInput").ap()
    ins["biascol"] = nc.dram_tensor(
        "biascol", [P, BPC * CN], F32, kind="ExternalInput").ap()
    ins["DTall"] = nc.dram_tensor(
        "DTall", [P, BPC * CN * ADIM], F32, kind="ExternalInput").ap()
    out_ap = nc.dram_tensor("act", [BPC, ADIM], F32, kind="ExternalOutput").ap()

    with tile.TileContext(nc) as tc:
        with ExitStack() as ctx:
            pools = make_pools(ctx, tc)
            kernel_body(ctx, tc, ins, out_ap, iters, pools, cm_const)
    nc.compile()
    return nc


def _to_rowspace(arr):
    """[B, N] -> [B, 128, NSLAB] row-space: row 32j holds n-slab j, rest 0."""
    B = arr.shape[0]
    out = np.zeros((B, P, NSLAB), np.float32)
    for j in range(CG):
        out[:, 32 * j, :] = arr[:, NSLAB * j:NSLAB * (j + 1)]
    return out


def prep_in_maps(obs, v0, tau, gain, bias, W, mask, E, D):
    f = np.float32
    obs, v0, tau, gain, bias, W, mask, E, D = [
        np.asarray(x, dtype=f) for x in (obs, v0, tau, gain, bias, W, mask, E, D)
    ]
    am = (DT / tau) * mask                    # [64, N]
    cm = (1.0 - DT / tau) * mask
    Wf = W * am[:, :, None] * mask[:, None, :]
    # permute the contraction dim to the block-transpose chunk map:
    # WT[b, p, k*N + n] = Wf[b, n, M_INDEX[p, k]]   (done per-core for memory)
    WT = np.empty((B_FULL, P, CN * N), BF16_NP)
    for b in range(B_FULL):
        WT[b] = Wf[b][:, M_INDEX].transpose(1, 2, 0).reshape(
            P, CN * N).astype(BF16_NP)
    ETp = np.ascontiguousarray(
        (E * am[:, :, None]).transpose(0, 2, 1)).astype(BF16_NP)  # [64, OBS, N]
    DTp = np.ascontiguousarray(D.transpose(0, 2, 1)).reshape(B_FULL, P, CN * ADIM)
    obsT = np.ascontiguousarray(obs.T).astype(BF16_NP)  # [OBS, 64]
    cm4 = _to_rowspace(cm)
    g4 = _to_rowspace(gain)
    bc4 = _to_rowspace(bias * (1.0 - cm))
    vs04 = _to_rowspace(v0 + bias)
    biascol = bias.reshape(B_FULL, P, CN)

    in_maps = []
    for core in range(NCORES):
        s = slice(core * BPC, (core + 1) * BPC)
        crow = np.empty((P, BPC * 3 * NSLAB), f)
        for i, b in enumerate(range(core * BPC, (core + 1) * BPC)):
            for k, arr in enumerate((cm4, g4, bc4)):
                crow[:, (i * 3 + k) * NSLAB:(i * 3 + k + 1) * NSLAB] = arr[b]
        vs0row = np.ascontiguousarray(
            vs04[s].transpose(1, 0, 2).reshape(P, BPC * NSLAB))
        bcol = np.ascontiguousarray(
            biascol[s].transpose(1, 0, 2).reshape(P, BPC * CN))
        et = np.ascontiguousarray(
            ETp[s].transpose(1, 0, 2).reshape(OBS, BPC * N))
        dtall = np.ascontiguousarray(
            DTp[s].transpose(1, 0, 2).reshape(P, BPC * CN * ADIM))
        in_maps.append({
            "WT": np.ascontiguousarray(WT[s]),
            "ETall": et,
            "obsT": np.ascontiguousarray(obsT[:, s]),
            "crow": crow,
            "vs0row": vs0row,
            "biascol": bcol,
            "DTall": dtall,
        })
    return in_maps


_NC_CACHE = {}


def _get_nc(cm_const=None):
    key = cm_const
    if key not in _NC_CACHE:
        _NC_CACHE[key] = build_nc(cm_const=cm_const)
    return _NC_CACHE[key]


def _detect_cm_const(tau, mask):
    tau = np.asarray(tau, np.float32)
    mask = np.asarray(mask, np.float32)
    if np.all(mask == 1.0) and np.all(tau == tau.flat[0]):
        return float(1.0 - DT / tau.flat[0])
    return None


def kernel(obs, v0, tau, gain, bias, W, mask, E, D):
    nc = _get_nc(_detect_cm_const(tau, mask))
    in_maps = prep_in_maps(obs, v0, tau, gain, bias, W, mask, E, D)
    res = run_bass_kernel_spmd(nc, in_maps, core_ids=list(range(NCORES)))
    return np.concatenate([res.results[c]["act"] for c in range(NCORES)], axis=0)
